# revision 1
# baseline (speedup 1.0000x reference)
"""TRN2 Bass kernel for nn_DeeperGCNLayerMix (GENConv softmax-aggr + MLP/BN/LN mix).

Self-contained: accepts FULL inputs, shards nodes across 8 NeuronCores
internally (SPMD, one NEFF), returns the FULL [50000, 128] output.

Strategy:
- Nodes sharded by dst range across 8 cores. Per-core edges bucketed by
  128-node dst window, split lo/hi by src (int16 dma_gather index limit),
  padded to 128-edge chunks; chunk structure equalized across cores so a
  single NEFF serves all 8 cores with per-core index/data streams.
- Edge phase per chunk: dma_gather x[src] (512B rows), ACT exp(t*g)->fp16,
  DVE relu / max(.,1) / mul, one-hot built on DVE via
  is_equal(iota, dst_local), two fp16 matmuls accumulate per-window
  [ch, nodes] PSUM: s = sum(e), u = sum(r*e).
- Softmax shift invariance removes the segment-max pass (logits bounded);
  agg = u/(s+1e-16), with the reference's +eps terms folded downstream
  (error ~1e-7, far below fp32-envelope tolerance).
- Node phase transposed (ch on partitions): h = agg + x + eps, h@W1 (fp32),
  global BatchNorm stats via bn_stats/bn_aggr + AllReduce of [128,4]
  partials, fused affine+relu (ACT), @W2 (fp16), PE transpose back to
  node-major, LayerNorm per node (bn_stats on free dim), final mix, DMA out.
"""

from contextlib import ExitStack
from dataclasses import dataclass, field

import numpy as np

import concourse.bacc as bacc
import concourse.mybir as mybir
import concourse.tile as tile
from concourse import bass_utils

F32 = mybir.dt.float32
F16 = mybir.dt.float16
I16 = mybir.dt.int16
AF = mybir.ActivationFunctionType
ALU = mybir.AluOpType

N = 50000
NC = 8
D = 128
W = 128
KCH = 16
NT = 512
EPS_MSG = 1e-7
BN_EPS = 1e-5
LN_EPS = 1e-5
BETA_L = 0.5


@dataclass
class Plan:
    N: int
    LO: int
    NSH: int = 0
    NW: int = 0
    NPAD: int = 0
    SBW: int = 7
    nch: list = field(default_factory=list)
    sbs: list = field(default_factory=list)
    chunk_cls: list = field(default_factory=list)
    chunk_w: list = field(default_factory=list)
    first_of_w: dict = field(default_factory=dict)
    last_of_w: dict = field(default_factory=dict)
    runs: list = field(default_factory=list)
    CT: int = 0
    CLO: int = 0
    CHI: int = 0

    def key(self):
        return (self.N, self.LO, tuple(map(tuple, self.nch)))


def make_plan(n, edge_index, LO=32768, SBW=7):
    src = np.asarray(edge_index[0]).astype(np.int64)
    dst = np.asarray(edge_index[1]).astype(np.int64)
    p = Plan(N=n, LO=LO, SBW=SBW)
    p.NSH = n // NC
    p.NW = (p.NSH + W - 1) // W
    p.NPAD = p.NW * W

    core = dst // p.NSH
    win = (dst % p.NSH) // W
    cls = (src >= LO).astype(np.int64)
    counts = np.zeros((NC, p.NW, 2), np.int64)
    np.add.at(counts, (core, win, cls), 1)
    chmax = np.ceil(counts / 128).astype(np.int64).max(axis=0)
    chmax[:, 0] = np.maximum(chmax[:, 0], 1)
    p.nch = chmax.tolist()

    for w0 in range(0, p.NW, SBW):
        p.sbs.append((w0, min(w0 + SBW, p.NW)))

    c_in_cls = [0, 0]
    for (w0, w1) in p.sbs:
        for c in (0, 1):
            g0 = len(p.chunk_cls)
            n_run = 0
            for w in range(w0, w1):
                for _ in range(p.nch[w][c]):
                    if w not in p.first_of_w:
                        p.first_of_w[w] = len(p.chunk_cls)
                    p.last_of_w[w] = len(p.chunk_cls)
                    p.chunk_cls.append(c)
                    p.chunk_w.append(w)
                    n_run += 1
            if n_run:
                p.runs.append((c, g0, c_in_cls[c], n_run))
            c_in_cls[c] += n_run
    p.CT = len(p.chunk_cls)
    p.CLO, p.CHI = c_in_cls[0], c_in_cls[1]
    return p


def make_core_inputs(p, x, edge_index, t, W1, b1, bn_gamma, bn_beta,
                     W2, b2, ln_gamma, ln_beta):
    x = np.ascontiguousarray(np.asarray(x, np.float32))
    src = np.asarray(edge_index[0]).astype(np.int64)
    dst = np.asarray(edge_index[1]).astype(np.int64)
    t = float(np.asarray(t))

    iota = np.broadcast_to(np.arange(128, dtype=np.float16), (128, 128)).copy()
    ident = np.eye(128, dtype=np.float32)
    lng_row = np.broadcast_to(
        (1.0 - BETA_L) * np.asarray(ln_gamma, np.float32), (128, 128)).copy()
    lnb_row = np.broadcast_to(
        (1.0 - BETA_L) * np.asarray(ln_beta, np.float32), (128, 128)).copy()

    vecs = np.zeros((128, 8), np.float32)
    vecs[:, 0] = t
    vecs[:, 1] = np.asarray(b2, np.float32)
    vecs[:, 2] = np.asarray(bn_gamma, np.float32)[0:128]
    vecs[:, 3] = np.asarray(bn_gamma, np.float32)[128:256]
    vecs[:, 4] = np.asarray(bn_beta, np.float32)[0:128]
    vecs[:, 5] = np.asarray(bn_beta, np.float32)[128:256]
    vecs[:, 6] = EPS_MSG

    W1m = np.ascontiguousarray(np.asarray(W1, np.float32))
    W2m = np.ascontiguousarray(np.asarray(W2, np.float32).astype(np.float16))

    def wrap16(ids):
        a = ids.reshape(-1, 16).T
        return np.tile(a, (8, 1)).copy()

    gmap = {}
    for g, (cl, w) in enumerate(zip(p.chunk_cls, p.chunk_w)):
        gmap.setdefault((w, cl), []).append(g)
    base = [0, 0]
    stream_base = np.zeros((p.NW, 2), np.int64)
    for w in range(p.NW):
        for cl in (0, 1):
            stream_base[w, cl] = base[cl]
            base[cl] += p.nch[w][cl]

    order = np.argsort(dst, kind="stable")
    src_s, dst_s = src[order], dst[order]
    in_maps = []
    for c in range(NC):
        lo_n, hi_n = c * p.NSH, (c + 1) * p.NSH
        a, b = np.searchsorted(dst_s, [lo_n, hi_n])
        s_c, d_c = src_s[a:b], dst_s[a:b]
        dloc = d_c - lo_n
        wloc = dloc // W
        m = (dloc % W).astype(np.float16)
        cls = (s_c >= p.LO).astype(np.int64)

        idx_lo = np.zeros(p.CLO * 128, np.int16)
        idx_hi = np.zeros(max(p.CHI, 1) * 128, np.int16)
        dstloc = np.full((128, p.CT), -1.0, np.float16)

        key = wloc * 2 + cls
        eorder = np.argsort(key, kind="stable")
        key_s = key[eorder]
        for w in range(p.NW):
            for cl in (0, 1):
                lo_i, hi_i = np.searchsorted(key_s, [w * 2 + cl, w * 2 + cl + 1])
                eids = eorder[lo_i:hi_i]
                n = len(eids)
                assert n <= p.nch[w][cl] * 128, (c, w, cl, n)
                if n == 0:
                    continue
                sb = int(stream_base[w, cl]) * 128
                if cl == 0:
                    idx_lo[sb:sb + n] = s_c[eids].astype(np.int16)
                else:
                    idx_hi[sb:sb + n] = (s_c[eids] - p.LO).astype(np.int16)
                glist = np.asarray(gmap[(w, cl)])
                rows = np.arange(n) % 128
                cols = glist[np.arange(n) // 128]
                dstloc[rows, cols] = m[eids]

        im = {
            "x": x,
            "xshard": np.pad(x[lo_n:hi_n], ((0, p.NPAD - p.NSH), (0, 0))),
            "idx_lo": wrap16(idx_lo),
            "idx_hi": wrap16(idx_hi),
            "dstloc": dstloc,
            "iota": iota,
            "ident": ident,
            "W1": W1m,
            "W2f16": W2m,
            "vecs": vecs,
            "lng_row": lng_row,
            "lnb_row": lnb_row,
        }
        in_maps.append(im)
    return in_maps


def input_specs(p):
    return {
        "x": ([p.N, D], F32),
        "xshard": ([p.NPAD, D], F32),
        "idx_lo": ([128, p.CLO * 8], I16),
        "idx_hi": ([128, max(p.CHI, 1) * 8], I16),
        "dstloc": ([128, p.CT], F16),
        "iota": ([128, 128], F16),
        "ident": ([128, 128], F32),
        "W1": ([128, 256], F32),
        "W2f16": ([256, 128], F16),
        "vecs": ([128, 8], F32),
        "lng_row": ([128, 128], F32),
        "lnb_row": ([128, 128], F32),
    }


def emit_kernel(ctx, tc, p, aps):
    nc = tc.nc
    NPAD, NW, NSH = p.NPAD, p.NW, p.NSH

    cpool = ctx.enter_context(tc.tile_pool(name="consts", bufs=1))
    idxt = [None, None]
    idxt[0] = cpool.tile([128, p.CLO * 8], I16, tag="idx_lo", name="idx_lo_t")
    nc.sync.dma_start(idxt[0][:], aps["idx_lo"][:])
    if p.CHI:
        idxt[1] = cpool.tile([128, p.CHI * 8], I16, tag="idx_hi", name="idx_hi_t")
        nc.sync.dma_start(idxt[1][:], aps["idx_hi"][:])
    dstloc = cpool.tile([128, p.CT], F16, tag="dstloc")
    nc.sync.dma_start(dstloc[:], aps["dstloc"][:])
    iota = cpool.tile([128, 128], F16, tag="iota")
    nc.sync.dma_start(iota[:], aps["iota"][:])
    ident = cpool.tile([128, 128], F32, tag="ident")
    nc.sync.dma_start(ident[:], aps["ident"][:])
    W1t = cpool.tile([128, 256], F32, tag="w1")
    nc.sync.dma_start(W1t[:], aps["W1"][:])
    W2t = [cpool.tile([128, 128], F16, tag=f"w2_{i}", name=f"w2t_{i}")
           for i in range(2)]
    nc.sync.dma_start(W2t[0][:], aps["W2f16"][0:128, :])
    nc.sync.dma_start(W2t[1][:], aps["W2f16"][128:256, :])
    vecs = cpool.tile([128, 8], F32, tag="vecs")
    nc.sync.dma_start(vecs[:], aps["vecs"][:])
    lng_row = cpool.tile([128, 128], F32, tag="lng")
    nc.sync.dma_start(lng_row[:], aps["lng_row"][:])
    lnb_row = cpool.tile([128, 128], F32, tag="lnb")
    nc.sync.dma_start(lnb_row[:], aps["lnb_row"][:])
    t_ap = vecs[:, 0:1]
    b2_ap = vecs[:, 1:2]

    swT = cpool.tile([128, NW * 256], F32, tag="swT")

    xlo = aps["x"][0:p.LO, :]
    xhi = aps["x"][p.LO:p.N, :]

    # ---- edge phase ----
    with tc.tile_pool(name="gat", bufs=3) as gp, \
         tc.tile_pool(name="vals", bufs=3) as vp, \
         tc.tile_pool(name="epsum", bufs=8, space="PSUM") as pp:
        psw = {}
        qn = [0]

        def do_call(cl, g0, c0, k):
            g = gp.tile([128, k, 128], F32, tag="g")
            nc.gpsimd.dma_gather(
                g[:], (xlo if cl == 0 else xhi),
                idxt[cl][:, c0 * 8:(c0 + k) * 8],
                num_idxs=k * 128, num_idxs_reg=k * 128, elem_size=128,
                single_packet=False, queue_num=qn[0])
            qn[0] = (qn[0] + 1) % 4
            v = vp.tile([128, k, 128], F16, tag="v")
            nc.scalar.activation(v[:], g[:], AF.Exp, bias=0.0, scale=t_ap)
            r = vp.tile([128, k, 128], F16, tag="r")
            nc.vector.tensor_scalar(r[:], g[:], 0.0, None, ALU.max)
            u = vp.tile([128, k, 128], F16, tag="u")
            nc.vector.tensor_tensor(u[:], r[:], v[:], op=ALU.mult)
            e = vp.tile([128, k, 128], F16, tag="e")
            nc.vector.tensor_scalar(e[:], v[:], 1.0, None, ALU.max)
            oh = vp.tile([128, k, 128], F16, tag="oh")
            nc.vector.tensor_tensor(
                oh[:],
                iota[:].unsqueeze(1).broadcast_to([128, k, 128]),
                dstloc[:, g0:g0 + k].unsqueeze(2).broadcast_to([128, k, 128]),
                op=ALU.is_equal)
            for jj in range(k):
                j = g0 + jj
                w = p.chunk_w[j]
                st = p.first_of_w[w] == j
                sp_ = p.last_of_w[w] == j
                if st:
                    psw[w] = pp.tile([128, 256], F32, tag="ps", name=f"psw_{w}")
                nc.tensor.matmul(psw[w][:, 0:128], e[:, jj, :], oh[:, jj, :],
                                 start=st, stop=sp_, skip_group_check=True)
                nc.tensor.matmul(psw[w][:, 128:256], u[:, jj, :], oh[:, jj, :],
                                 start=False, stop=sp_, skip_group_check=True)
                if sp_:
                    nc.scalar.copy(swT[:, w * 256:(w + 1) * 256], psw[w][:])
                    del psw[w]

        for (cl, g0, c0, n_run) in p.runs:
            off = 0
            while off < n_run:
                k = min(KCH, n_run - off)
                do_call(cl, g0 + off, c0 + off, k)
                off += k

    # ---- node phase ----
    np3 = ctx.enter_context(tc.tile_pool(name="node3", bufs=1))
    dramp = ctx.enter_context(tc.tile_pool(name="dram", bufs=1, space="DRAM"))
    swv = swT[:].rearrange("p (w q) -> p w q", q=256)

    with tc.tile_pool(name="tpsum", bufs=2, space="PSUM") as tp, \
         tc.tile_pool(name="scr", bufs=2) as sp:
        xT = np3.tile([128, NPAD], F32, tag="X")
        for w in range(NW):
            xin = sp.tile([128, 128], F32, tag="xin")
            nc.sync.dma_start(xin[:], aps["xshard"][w * 128:(w + 1) * 128, :])
            ps = tp.tile([128, 128], F32, tag="pst")
            nc.tensor.transpose(ps[:], xin[:], ident[:])
            nc.scalar.activation(xT[:, w * 128:(w + 1) * 128], ps[:],
                                 AF.Identity, bias=vecs[:, 6:7], scale=1.0)

        spk = np3.tile([128, NW, 128], F32, tag="A")
        nc.vector.tensor_scalar(spk[:], swv[:, :, 0:128], 1e-16, None, ALU.add)
        rcp = np3.tile([128, NW, 128], F32, tag="B")
        nc.vector.reciprocal(rcp[:], spk[:])
        h = np3.tile([128, NW, 128], F32, tag="A")
        nc.vector.tensor_tensor(h[:], swv[:, :, 128:256], rcp[:], op=ALU.mult)
        hf = h[:].rearrange("p w q -> p (w q)")
        nc.vector.tensor_tensor(hf, hf, xT[:], op=ALU.add)

        h1 = [np3.tile([128, NPAD], F16, tag=f"H{i}", name=f"h1_{i}")
              for i in range(2)]
        ntiles = []
        o = 0
        while o < NPAD:
            ntiles.append((o, min(NT, NPAD - o)))
            o += NT
        ng = 1
        for k in range(max(1, (NSH + 511) // 512), NSH + 1):
            if NSH % k == 0 and NSH // k <= 512:
                ng = k
                break
        gsz = NSH // ng
        stt = [(i * gsz, gsz) for i in range(ng)]
        partials = np3.tile([128, 4], F32, tag="partials")
        for ch in (0, 1):
            for (o, sz) in ntiles:
                ps = tp.tile([128, NT], F32, tag="psmm")
                nc.tensor.matmul(ps[:, 0:sz], W1t[:, ch * 128:(ch + 1) * 128],
                                 hf[:, o:o + sz], start=True, stop=True)
                nc.scalar.copy(h1[ch][:, o:o + sz], ps[:, 0:sz])
            stb = sp.tile([128, len(stt) * 6], F32, tag="stb")
            for i, (o, sz) in enumerate(stt):
                nc.vector.bn_stats(stb[:, i * 6:(i + 1) * 6], h1[ch][:, o:o + sz])
            mv = sp.tile([128, 2], F32, tag="mv")
            nc.vector.bn_aggr(mv[:], stb[:])
            msq = sp.tile([128, 1], F32, tag="msq")
            nc.vector.tensor_tensor(msq[:], mv[:, 0:1], mv[:, 0:1], op=ALU.mult)
            nc.vector.tensor_copy(partials[:, ch:ch + 1], mv[:, 0:1])
            nc.vector.tensor_tensor(partials[:, 2 + ch:3 + ch], mv[:, 1:2],
                                    msq[:], op=ALU.add)

        ib = dramp.tile([128, 4], F32, tag="ib")
        ob = dramp.tile([128, 4], F32, tag="ob")
        nc.sync.dma_start(ib[:], partials[:])
        nc.gpsimd.collective_compute(
            "AllReduce", ALU.add, replica_groups=[list(range(NC))],
            ins=[ib[:].opt()], outs=[ob[:].opt()])
        gst = sp.tile([128, 4], F32, tag="gst")
        nc.sync.dma_start(gst[:], ob[:])

        mg = sp.tile([128, 2], F32, tag="mg")
        nc.vector.tensor_scalar(mg[:], gst[:, 0:2], 1.0 / NC, None, ALU.mult)
        var = sp.tile([128, 2], F32, tag="var")
        nc.vector.tensor_tensor(var[:], mg[:], mg[:], op=ALU.mult)
        ex2 = sp.tile([128, 2], F32, tag="ex2")
        nc.vector.tensor_scalar(ex2[:], gst[:, 2:4], 1.0 / NC, None, ALU.mult)
        nc.vector.tensor_tensor(var[:], ex2[:], var[:], op=ALU.subtract)
        nc.vector.tensor_scalar(var[:], var[:], float(BN_EPS), None, ALU.add)
        rcv = sp.tile([128, 2], F32, tag="rcv")
        nc.vector.reciprocal(rcv[:], var[:])
        rstd = sp.tile([128, 2], F32, tag="rstd")
        nc.scalar.sqrt(rstd[:], rcv[:])
        aaf = sp.tile([128, 2], F32, tag="aaf")
        nc.vector.tensor_tensor(aaf[:], vecs[:, 2:4], rstd[:], op=ALU.mult)
        baf = sp.tile([128, 2], F32, tag="baf")
        nc.vector.tensor_tensor(baf[:], mg[:], aaf[:], op=ALU.mult)
        nc.vector.tensor_tensor(baf[:], vecs[:, 4:6], baf[:], op=ALU.subtract)

        for ch in (0, 1):
            nc.scalar.activation(h1[ch][:], h1[ch][:], AF.Relu,
                                 bias=baf[:, ch:ch + 1], scale=aaf[:, ch:ch + 1])
        yT = np3.tile([128, NPAD], F32, tag="X")
        for (o, sz) in ntiles:
            ps = tp.tile([128, NT], F32, tag="psy")
            nc.tensor.matmul(ps[:, 0:sz], W2t[0][:], h1[0][:, o:o + sz],
                             start=True, stop=False)
            nc.tensor.matmul(ps[:, 0:sz], W2t[1][:], h1[1][:, o:o + sz],
                             start=False, stop=True)
            nc.scalar.activation(yT[:, o:o + sz], ps[:, 0:sz], AF.Identity,
                                 bias=b2_ap, scale=1.0)

        yN = np3.tile([128, NPAD], F32, tag="A")
        mvall = np3.tile([128, NW * 2], F32, tag="mvall")
        for w in range(NW):
            ps = tp.tile([128, 128], F32, tag="pst")
            nc.tensor.transpose(ps[:], yT[:, w * 128:(w + 1) * 128], ident[:])
            nc.scalar.copy(yN[:, w * 128:(w + 1) * 128], ps[:])
            st6 = sp.tile([128, 6], F32, tag="st6")
            nc.vector.bn_stats(st6[:], yN[:, w * 128:(w + 1) * 128])
            nc.vector.bn_aggr(mvall[:, w * 2:(w + 1) * 2], st6[:])
        mvv = mvall[:].rearrange("p (w q) -> p w q", q=2)
        varn = np3.tile([128, NW, 1], F32, tag="varn")
        nc.vector.tensor_scalar(varn[:], mvv[:, :, 1:2], float(LN_EPS), None,
                                ALU.add)
        rcn = np3.tile([128, NW, 1], F32, tag="rcn")
        nc.vector.reciprocal(rcn[:], varn[:])
        rsn = np3.tile([128, NW, 1], F32, tag="rsn")
        nc.scalar.sqrt(rsn[:], rcn[:])

        zc = np3.tile([128, NW, 128], F32, tag="B")
        yNv = yN[:].rearrange("p (w q) -> p w q", q=128)
        nc.vector.tensor_tensor(zc[:], yNv,
                                mvv[:, :, 0:1].broadcast_to([128, NW, 128]),
                                op=ALU.subtract)
        nc.vector.tensor_tensor(zc[:], zc[:],
                                rsn[:].broadcast_to([128, NW, 128]),
                                op=ALU.mult)
        nc.vector.tensor_tensor(zc[:], zc[:],
                                lng_row[:].unsqueeze(1).broadcast_to(
                                    [128, NW, 128]), op=ALU.mult)
        nc.vector.tensor_tensor(zc[:], zc[:],
                                lnb_row[:].unsqueeze(1).broadcast_to(
                                    [128, NW, 128]), op=ALU.add)
        zf = zc[:].rearrange("p w q -> p (w q)")
        rz = np3.tile([128, NPAD], F32, tag="A")
        nc.scalar.activation(rz[:], zf, AF.Relu)
        nc.vector.tensor_tensor(rz[:], rz[:], zf, op=ALU.add)
        for w in range(NW):
            xin = sp.tile([128, 128], F32, tag="xin")
            nc.sync.dma_start(xin[:], aps["xshard"][w * 128:(w + 1) * 128, :])
            xh = sp.tile([128, 128], F32, tag="xh")
            nc.vector.tensor_scalar(xh[:], xin[:], 1.0 - BETA_L, None, ALU.mult)
            nc.vector.tensor_tensor(rz[:, w * 128:(w + 1) * 128],
                                    rz[:, w * 128:(w + 1) * 128], xh[:],
                                    op=ALU.add)

        nc.sync.dma_start(
            aps["yout"][:].rearrange("(w q) c -> q w c", q=128),
            rz[:].rearrange("p (w q) -> p w q", q=128))


_cache = {}


def _get_compiled(p):
    key = p.key()
    if key in _cache:
        return _cache[key]
    nc = bacc.Bacc("TRN2", target_bir_lowering=False, debug=False,
                   num_devices=NC, num_swdge_queues=4)
    aps = {}
    for name, (shape, dt) in input_specs(p).items():
        aps[name] = nc.dram_tensor(name, shape, dt, kind="ExternalInput").ap()
    aps["yout"] = nc.dram_tensor("yout", [p.NPAD, 128], F32,
                                 kind="ExternalOutput").ap()
    with tile.TileContext(nc) as tc:
        with ExitStack() as ctx:
            emit_kernel(ctx, tc, p, aps)
    nc.compile()
    _cache[key] = nc
    return nc


def kernel(x, edge_index, t, W1, b1, bn_gamma, bn_beta, W2, b2,
           ln_gamma, ln_beta):
    x = np.asarray(x)
    edge_index = np.asarray(edge_index)
    p = make_plan(x.shape[0], edge_index)
    ims = make_core_inputs(p, x, edge_index, t, W1, b1, bn_gamma, bn_beta,
                           W2, b2, ln_gamma, ln_beta)
    nc = _get_compiled(p)
    res = bass_utils.run_bass_kernel_spmd(nc, ims, core_ids=list(range(NC)))
    out = np.concatenate([res.results[c]["yout"][:p.NSH] for c in range(NC)])
    return out.astype(np.float32)

